# revision 28
# baseline (speedup 1.0000x reference)
"""ConvProduct forward (one-hot 2x2/stride-2 conv) as a Bass/Tile kernel on 8 trn2 cores.

Pure data parallel over batch (8 batches/core). Per batch:
  - x is pre-cast to bf16 on host (input prep, like the one-hot W build);
    all 8 loads are issued upfront on the two HWDGE rings (one DMA per kh so
    the outermost AP dim 64 spreads across the SDMA engines) so no load ever
    queues behind a store in the ring FIFO.
  - one DVE 32x32 block-transpose Q -> T (bf16) with block-permuted source:
    T col-block (c*4+wg) is the in-block transpose of Q col-block
    B = 16*wg+c (= wo); T[32A+i, .] with A = kh*2+a holds
    x[b, 2*(32a+j)+kh, (2B+kw)*16+cin], i = kw*16+cin.
  - per c: ONE bf16 matmul, full-array stationary lhsT = T[:, c*128:(c+1)*128]
    (m = wg*32+j, pixel (ho=a*32+j, wo=16wg+c)), moving rhs = W2 [128, 512]
    whose column halves are the a=0 / a=1 masked one-hot gathers. The K=128
    contraction covers both kh strips and zero-masks the wrong a-half, so
    psum[m, a*256+o] is complete in one N=512 matmul; the 128-col LDWEIGHTS is
    amortized over both ho-halves.
  - evacuation: psum c-pair -> st_a[:, cp*512:...] with a strided-src
    [128,512] copy, alternating ScalarE/VectorE.
  - store: per (b, a, wg) one DMA [j=32, (c o)]: every partition is one fully
    contiguous run and the outermost dim (32) spreads descriptors across the
    16 SDMA engines; stores rotate over the sync/scalar HWDGE rings plus the
    gpsimd SWDGE ring. The first and last batches are processed in half-batch
    chunks (8 c-groups) to shorten the pipeline ramp and the final drain.
"""
import numpy as np

B, H, Wd, Cin = 64, 128, 128, 16
KH, KW, Cout = 2, 2, 256
Ho, Wo = 64, 64
NCORES = 8
BPC = B // NCORES

_CACHE = {}


def _build_nc():
    import concourse.bass as bass
    import concourse.mybir as mybir
    import concourse.tile as tile
    from concourse import bacc

    f32 = mybir.dt.float32
    bf16 = mybir.dt.bfloat16
    nc = bacc.Bacc("TRN2", target_bir_lowering=False, debug=False)

    x = nc.dram_tensor("x", [BPC, H, Wd, Cin], bf16, kind="ExternalInput")
    w = nc.dram_tensor("w", [128, 2 * Cout], bf16, kind="ExternalInput")
    out = nc.dram_tensor("out", [BPC, Ho, Wo, Cout], f32, kind="ExternalOutput")

    with tile.TileContext(nc) as tc:
        with (
            tc.tile_pool(name="wp", bufs=1) as wp,
            tc.tile_pool(name="qp", bufs=BPC) as qp,
            tc.tile_pool(name="tp", bufs=3) as tp,
            tc.tile_pool(name="sp", bufs=3) as sp,
            tc.tile_pool(name="sph", bufs=2) as sph,
            tc.tile_pool(name="pp", bufs=4, space="PSUM") as pp,
        ):
            w_sb = wp.tile([128, 2 * Cout], bf16)
            nc.gpsimd.dma_start(w_sb[:], w.ap())

            qs = []
            for b in range(BPC):
                q = qp.tile([128, Wd * Cin], bf16, tag="q", name=f"q_{b}")
                src = x.ap()[b].rearrange("(ho kh) w c -> kh ho (w c)", kh=2)
                if b == 0:
                    # batch 0: split each kh-half across both rings so the
                    # first transpose can start sooner
                    nc.sync.dma_start(q[0:32, :], src[0][0:32])
                    nc.scalar.dma_start(q[32:64, :], src[0][32:64])
                    nc.sync.dma_start(q[64:96, :], src[1][0:32])
                    nc.scalar.dma_start(q[96:128, :], src[1][32:64])
                else:
                    nc.sync.dma_start(q[0:64, :], src[0])
                    nc.scalar.dma_start(q[64:128, :], src[1])
                qs.append(q)

            for b in range(BPC):
                q = qs[b]
                t = tp.tile([128, Wd * Cin], bf16, tag="t")
                qperm = q[:].rearrange("p (wg c e) -> p c wg e", wg=4, c=16, e=32)
                tv = t[:].rearrange("p (c wg e) -> p c wg e", wg=4, c=16, e=32)
                nc.vector.transpose(tv, qperm)

                dstv = out.ap()[b].rearrange(
                    "(a j) (wg c) o -> a wg j c o", a=2, wg=4
                )
                if b == 0:
                    chunks = [(0, 4), (4, 8), (8, 12), (12, 16)]
                elif b == BPC - 1:
                    chunks = [(0, 8), (8, 16)]
                else:
                    chunks = [(0, 16)]
                for lo, hi in chunks:
                    wdt = hi - lo
                    pool, tagsuf = (sp, "") if wdt == 16 else (sph, "h")
                    sts = [
                        pool.tile([128, wdt * Cout], f32, tag=f"st{a}{tagsuf}",
                                  name=f"st{a}_{b}_{lo}")
                        for a in range(2)
                    ]
                    for cp in range(wdt // 2):
                        pt = pp.tile([128, 1024], f32, tag="ps")
                        for cc in range(2):
                            c = lo + 2 * cp + cc
                            nc.tensor.matmul(
                                pt[:, cc * 512:(cc + 1) * 512],
                                t[:, c * 128:(c + 1) * 128],
                                w_sb[:],
                                start=True,
                                stop=True,
                            )
                        # evac: (c0,c1) x one a-half per op, strided src over
                        # the two banks
                        pv = pt[:].rearrange("p (cc a o) -> p a cc o", cc=2, a=2)
                        for a in range(2):
                            stsl = sts[a][:, cp * 512:(cp + 1) * 512].rearrange(
                                "p (cc o) -> p cc o", cc=2
                            )
                            if (cp + a) % 2 == 0:
                                nc.scalar.copy(stsl, pv[:, a])
                            else:
                                nc.vector.tensor_copy(stsl, pv[:, a])

                    for a in range(2):
                        for wg in range(4):
                            # balance total queue bytes: the HWDGE rings also
                            # carry the loads, so the SWDGE ring takes 3 of 8
                            # stores (wg3 for both a, plus alternating wg2)
                            if wg == 3 or (wg == 2 and a == (b % 2)):
                                eng = nc.gpsimd
                            else:
                                eng = nc.sync if a == 0 else nc.scalar
                            eng.dma_start(
                                dstv[a][wg][:, lo:hi, :],
                                sts[a][wg * 32:(wg + 1) * 32, :],
                            )

    nc.compile()
    return nc


def _get_nc():
    if "v" not in _CACHE:
        _CACHE["v"] = _build_nc()
    return _CACHE["v"]


def _build_w(kernel_idx: np.ndarray) -> np.ndarray:
    import ml_dtypes

    kidx = np.asarray(kernel_idx).astype(np.int64)
    w = np.zeros((128, 2 * Cout), np.float32)
    o = np.arange(Cout)
    for kh in range(KH):
        for a in range(2):
            for kw in range(KW):
                w[kh * 64 + a * 32 + kw * 16 + kidx[kh, kw], a * Cout + o] = 1.0
    return w.astype(ml_dtypes.bfloat16)


def kernel(x: np.ndarray, kernel_idx: np.ndarray) -> np.ndarray:
    import ml_dtypes
    from concourse.bass_utils import run_bass_kernel_spmd

    x = np.ascontiguousarray(
        np.asarray(x, dtype=np.float32).astype(ml_dtypes.bfloat16)
    )
    w = _build_w(kernel_idx)
    nc = _get_nc()

    in_maps = [
        {"x": x[c * BPC:(c + 1) * BPC], "w": w} for c in range(NCORES)
    ]
    res = run_bass_kernel_spmd(nc, in_maps, core_ids=list(range(NCORES)))
    kernel.last_results = res
    return np.concatenate([res.results[c]["out"] for c in range(NCORES)], axis=0)


# revision 30
# speedup vs baseline: 1.0441x; 1.0441x over previous
"""ConvProduct forward (one-hot 2x2/stride-2 conv) as a Bass/Tile kernel on 8 trn2 cores.

Pure data parallel over batch (8 batches/core). Per batch:
  - x is pre-cast to bf16 on host (input prep, like the one-hot W build);
    all 8 loads are issued upfront on the two HWDGE rings (one DMA per kh so
    the outermost AP dim 64 spreads across the SDMA engines) so no load ever
    queues behind a store in the ring FIFO.
  - one DVE 32x32 block-transpose Q -> T (bf16) with block-permuted source:
    T col-block (c*4+wg) is the in-block transpose of Q col-block
    B = 16*wg+c (= wo); T[32A+i, .] with A = kh*2+a holds
    x[b, 2*(32a+j)+kh, (2B+kw)*16+cin], i = kw*16+cin.
  - per c: ONE bf16 matmul, full-array stationary lhsT = T[:, c*128:(c+1)*128]
    (m = wg*32+j, pixel (ho=a*32+j, wo=16wg+c)), moving rhs = W2 [128, 512]
    whose column halves are the a=0 / a=1 masked one-hot gathers. The K=128
    contraction covers both kh strips and zero-masks the wrong a-half, so
    psum[m, a*256+o] is complete in one N=512 matmul; the 128-col LDWEIGHTS is
    amortized over both ho-halves.
  - evacuation: psum c-pair -> st_a[:, cp*512:...] with a strided-src
    [128,512] copy, alternating ScalarE/VectorE.
  - store: per (b, a, wg) one DMA [j=32, (c o)]: every partition is one fully
    contiguous run and the outermost dim (32) spreads descriptors across the
    16 SDMA engines; stores rotate over the sync/scalar HWDGE rings plus the
    gpsimd SWDGE ring. The first and last batches are processed in half-batch
    chunks (8 c-groups) to shorten the pipeline ramp and the final drain.
"""
import numpy as np

B, H, Wd, Cin = 64, 128, 128, 16
KH, KW, Cout = 2, 2, 256
Ho, Wo = 64, 64
NCORES = 8
BPC = B // NCORES

_CACHE = {}


def _build_nc():
    import concourse.bass as bass
    import concourse.mybir as mybir
    import concourse.tile as tile
    from concourse import bacc

    f32 = mybir.dt.float32
    bf16 = mybir.dt.bfloat16
    nc = bacc.Bacc("TRN2", target_bir_lowering=False, debug=False)

    x = nc.dram_tensor("x", [BPC, H, Wd, Cin], bf16, kind="ExternalInput")
    w = nc.dram_tensor("w", [128, 2 * Cout], bf16, kind="ExternalInput")
    out = nc.dram_tensor("out", [BPC, Ho, Wo, Cout], f32, kind="ExternalOutput")

    with tile.TileContext(nc) as tc:
        with (
            tc.tile_pool(name="wp", bufs=1) as wp,
            tc.tile_pool(name="qp", bufs=BPC) as qp,
            tc.tile_pool(name="tp", bufs=3) as tp,
            tc.tile_pool(name="sp", bufs=3) as sp,
            tc.tile_pool(name="sph", bufs=2) as sph,
            tc.tile_pool(name="pp", bufs=4, space="PSUM") as pp,
        ):
            w_sb = wp.tile([128, 2 * Cout], bf16)
            nc.gpsimd.dma_start(w_sb[:], w.ap())

            qs = []
            for b in range(BPC):
                q = qp.tile([128, Wd * Cin], bf16, tag="q", name=f"q_{b}")
                src = x.ap()[b].rearrange("(ho kh) w c -> kh ho (w c)", kh=2)
                nc.sync.dma_start(q[0:64, :], src[0])
                nc.scalar.dma_start(q[64:128, :], src[1])
                qs.append(q)

            for b in range(BPC):
                q = qs[b]
                t = tp.tile([128, Wd * Cin], bf16, tag="t")
                qperm = q[:].rearrange("p (wg c e) -> p c wg e", wg=4, c=16, e=32)
                tv = t[:].rearrange("p (c wg e) -> p c wg e", wg=4, c=16, e=32)
                nc.vector.transpose(tv, qperm)

                dstv = out.ap()[b].rearrange(
                    "(a j) (wg c) o -> a wg j c o", a=2, wg=4
                )
                chunks = [(0, 8), (8, 16)] if b in (0, BPC - 1) else [(0, 16)]
                for lo, hi in chunks:
                    wdt = hi - lo
                    pool, tagsuf = (sp, "") if wdt == 16 else (sph, "h")
                    sts = [
                        pool.tile([128, wdt * Cout], f32, tag=f"st{a}{tagsuf}",
                                  name=f"st{a}_{b}_{lo}")
                        for a in range(2)
                    ]
                    for cp in range(wdt // 2):
                        pt = pp.tile([128, 1024], f32, tag="ps")
                        for cc in range(2):
                            c = lo + 2 * cp + cc
                            nc.tensor.matmul(
                                pt[:, cc * 512:(cc + 1) * 512],
                                t[:, c * 128:(c + 1) * 128],
                                w_sb[:],
                                start=True,
                                stop=True,
                            )
                        # evac: (c0,c1) x one a-half per op, strided src over
                        # the two banks
                        pv = pt[:].rearrange("p (cc a o) -> p a cc o", cc=2, a=2)
                        for a in range(2):
                            stsl = sts[a][:, cp * 512:(cp + 1) * 512].rearrange(
                                "p (cc o) -> p cc o", cc=2
                            )
                            if (cp + a) % 2 == 0:
                                nc.scalar.copy(stsl, pv[:, a])
                            else:
                                nc.vector.tensor_copy(stsl, pv[:, a])

                    for a in range(2):
                        for wg in range(4):
                            # spread stores over all three DMA queues: the two
                            # HWDGE rings plus the SWDGE ring
                            if wg == 3:
                                eng = nc.gpsimd
                            else:
                                eng = nc.sync if a == 0 else nc.scalar
                            eng.dma_start(
                                dstv[a][wg][:, lo:hi, :],
                                sts[a][wg * 32:(wg + 1) * 32, :],
                            )

    nc.compile()
    return nc


def _get_nc():
    if "v" not in _CACHE:
        _CACHE["v"] = _build_nc()
    return _CACHE["v"]


def _build_w(kernel_idx: np.ndarray) -> np.ndarray:
    import ml_dtypes

    kidx = np.asarray(kernel_idx).astype(np.int64)
    w = np.zeros((128, 2 * Cout), np.float32)
    o = np.arange(Cout)
    for kh in range(KH):
        for a in range(2):
            for kw in range(KW):
                w[kh * 64 + a * 32 + kw * 16 + kidx[kh, kw], a * Cout + o] = 1.0
    return w.astype(ml_dtypes.bfloat16)


def kernel(x: np.ndarray, kernel_idx: np.ndarray) -> np.ndarray:
    import ml_dtypes
    from concourse.bass_utils import run_bass_kernel_spmd

    x = np.ascontiguousarray(
        np.asarray(x, dtype=np.float32).astype(ml_dtypes.bfloat16)
    )
    w = _build_w(kernel_idx)
    nc = _get_nc()

    in_maps = [
        {"x": x[c * BPC:(c + 1) * BPC], "w": w} for c in range(NCORES)
    ]
    res = run_bass_kernel_spmd(nc, in_maps, core_ids=list(range(NCORES)))
    kernel.last_results = res
    return np.concatenate([res.results[c]["out"] for c in range(NCORES)], axis=0)
